# revision 18
# baseline (speedup 1.0000x reference)
"""Trainium2 Bass kernel for nn_AligningModel (mel/phoneme GLU encoders + soft attention).

Strategy (v2):
  - Data-parallel over batch: 32 samples -> 8 cores x 4 slots, length-sorted so
    each slot's compile-time bound is tight (slot j holds sorted ranks 8j..8j+7).
  - bf16 datapath everywhere; fp8e4m3 DoubleRow matmuls (256-contract/pass) for
    the mel GLU gate path and the entire phoneme encoder (error-tolerant paths,
    validated via numpy: ~7e-3 final rel err vs 2e-2 gate). Mel a-path stays bf16.
  - Ping-pong y tiles per GLU block: conv reads the old tile, the residual add
    writes the new one -> no masked input copy.  Masking is a narrow post-mask
    on cols [min_len_in_group+1, W+2) only (cols below are valid in all cores).
  - All conv weights SBUF-resident (bf16/fp8, ~3.5MB), DMA'd once per-block so
    block 0's weights land before the first GLU needs them.
  - Weight reuse: mel convs stream chunk-pairs per stationary load; phoneme
    convs stream both slots of a pair per load (hides 256-col DoubleRow LDW).
  - Scale folding: sqrt(0.5)^b folded into g-path conv weights; softmax uses
    logits = 2*C^8*dots - C^8*ph_sq (mel_sq dropped: softmax-invariant),
    no max-subtraction, phoneme -1e9 mask folded into per-partition exp bias.
  - Z (softmax denominator) via ones-columns appended to time-major ph tiles.
  - Attention emitted interleaved across slot pairs (PE dots ping-pong with
    scalar exp) with mel_out identity-matmuls as PE filler to stay HAM-warm.
"""

import os
import numpy as np
import ml_dtypes

B = 32
N_CORES = 8
SPC = 4           # samples (slots) per core
T_MEL = 2000
MEL_D = 80
D = 256
C = float(np.sqrt(0.5))
C4 = 0.25         # C**4 exact
C8 = 0.0625       # C**8 exact

BF = ml_dtypes.bfloat16
F8 = ml_dtypes.float8_e4m3

_prog_cache = {}


def _chunks(total, cap=512):
    out = []
    off = 0
    while off < total:
        w = min(cap, total - off)
        out.append((off, w))
        off += w
    return out


def _pairs(lst):
    return [lst[i:i + 2] for i in range(0, len(lst), 2)]


def _ceil(x, m):
    return -(-x // m) * m


def _host_prep(mels, phonemes, mel_lens, phoneme_lens, embedding,
               mel_conv_w, mel_conv_b, ph_w, ph_b, mel_w, mel_b, S_pad):
    """Build the per-core input maps (numpy only). Returns (in_maps, perm,
    L, SL, LO, LOPH) where perm[8*j + c] = original sample index in core c
    slot j."""
    f32 = np.float32
    SP2 = S_pad + 2

    order = np.argsort(-np.asarray(mel_lens), kind="stable")
    perm = np.asarray(order)
    L = tuple(int(mel_lens[perm[8 * j]]) for j in range(SPC))
    SL = tuple(int(max(phoneme_lens[perm[8 * j + c]] for c in range(8)))
               for j in range(SPC))
    LO = tuple(int(min(mel_lens[perm[8 * j + c]] for c in range(8)))
               for j in range(SPC))
    LOPH = tuple(int(min(phoneme_lens[perm[8 * j + c]] for c in range(8)))
                 for j in range(SPC))

    assert not np.any(mel_conv_b) and not np.any(mel_b) and not np.any(ph_b), \
        "nonzero conv biases not supported by this kernel variant"

    # conv0 weights: [i, k, o] bf16
    w0h = np.ascontiguousarray(
        np.transpose(mel_conv_w.astype(f32), (1, 2, 0))).astype(BF)

    scale = (C ** np.arange(4, dtype=np.float64)).astype(f32)

    def enc_layout(w4):
        # w4: [4, O, 256, 3] -> [128(ci), 4(b), 3(k), 2(icb), O]
        t = np.transpose(w4, (2, 0, 3, 1))          # [i, b, k, o]
        t = t.reshape(2, 128, 4, 3, w4.shape[1])    # [icb, ci, b, k, o]
        return np.ascontiguousarray(np.transpose(t, (1, 2, 3, 0, 4)))

    wa = enc_layout(mel_w[:, :256].astype(f32)).astype(BF)
    wg = enc_layout(mel_w[:, 256:].astype(f32)
                    * scale[:, None, None, None]).astype(F8)
    wpf = ph_w.astype(f32).copy()
    wpf[:, 256:] *= scale[:, None, None, None]
    wp = enc_layout(wpf).astype(F8)

    id1 = np.eye(128, dtype=f32).astype(BF)
    idc4 = (C4 * np.eye(128, dtype=f32)).astype(BF)

    shared = {"w0": w0h, "wam": wa, "wgm": wg, "wph": wp,
              "id1": id1, "idc4": idc4}

    ar = np.arange(T_MEL)
    ars = np.arange(S_pad)
    in_maps = []
    for c in range(N_CORES):
        idx = [int(perm[8 * j + c]) for j in range(SPC)]
        m = dict(shared)
        mcm = np.zeros((SPC, MEL_D, T_MEL + 4), BF)
        vm = np.zeros((SPC, T_MEL + 4), BF)
        zph = np.zeros((SPC, 2, 128, S_pad + 4), BF)
        vph = np.zeros((SPC, S_pad + 4), BF)
        mv = np.full((SPC, S_pad), -1e9, f32)
        for j, b in enumerate(idx):
            mcm[j, :, 2:T_MEL + 2] = np.asarray(mels[b], f32).T.astype(BF)
            vm[j, 2:T_MEL + 2] = (ar < int(mel_lens[b])).astype(BF)
            pl = int(phoneme_lens[b])
            ph_pad = np.concatenate([[0], np.asarray(phonemes[b], np.int64)])[:S_pad]
            e = embedding[ph_pad].astype(f32)
            valid = (ars[:len(e)] <= pl)
            e[~valid] = 0.0
            zph[j, :, :, 2:2 + len(e)] = e.T.reshape(2, 128, len(e)).astype(BF)
            vph[j, 2:2 + len(e)] = valid.astype(BF)
            mv[j, :len(e)][valid] = 0.0
        m["mels_cm"] = mcm
        m["valid_mel"] = vm
        m["zph0"] = zph
        m["valid_ph"] = vph
        m["mvec"] = mv
        in_maps.append(m)
    return in_maps, perm, L, SL, LO, LOPH


def _build_program(S_pad, L, SL, LO, LOPH):
    from contextlib import ExitStack
    import concourse.bass as bass
    import concourse.bacc as bacc
    import concourse.tile as tile
    from concourse import mybir

    f32 = mybir.dt.float32
    bf16 = mybir.dt.bfloat16
    f8 = mybir.dt.float8e4
    AF = mybir.ActivationFunctionType
    ALU = mybir.AluOpType
    AX = mybir.AxisListType
    DR = mybir.MatmulPerfMode.DoubleRow
    SZ = S_pad + 4                 # z tile width (data at col offset 2)
    SZQ = _ceil(SZ, 16)

    # per-slot compile-time bounds
    W = [min(T_MEL, _ceil(L[j] + 2, 4)) for j in range(SPC)]       # mel conv cols
    Tb = [min(T_MEL, _ceil(L[j] + 2, 128)) for j in range(SPC)]    # attn rows
    SW = [min(S_pad, _ceil(SL[j] + 2, 4)) for j in range(SPC)]     # ph conv cols
    NSB = [min(S_pad // 128, _ceil(SL[j] + 2, 128) // 128) for j in range(SPC)]
    WQ = [_ceil(W[j] + 4, 16) for j in range(SPC)]
    YW = [Tb[j] + 4 for j in range(SPC)]
    mel_chunks = [_chunks(W[j]) for j in range(SPC)]
    ph_chunks = [_chunks(SW[j]) for j in range(SPC)]
    dot_chunks = [_chunks(Tb[j]) for j in range(SPC)]
    # narrow post-mask regions (first possibly-invalid data col .. end of data)
    MLO = [min(LO[j] + 2, W[j] + 2) for j in range(SPC)]
    PLO = [min(LOPH[j] + 3, SW[j] + 2) for j in range(SPC)]

    nc = bacc.Bacc()
    t_mcm = nc.dram_tensor("mels_cm", [SPC, MEL_D, T_MEL + 4], bf16, kind="ExternalInput")
    t_vm = nc.dram_tensor("valid_mel", [SPC, T_MEL + 4], bf16, kind="ExternalInput")
    t_zph = nc.dram_tensor("zph0", [SPC, 2, 128, SZ], bf16, kind="ExternalInput")
    t_vph = nc.dram_tensor("valid_ph", [SPC, SZ], bf16, kind="ExternalInput")
    t_mv = nc.dram_tensor("mvec", [SPC, S_pad], f32, kind="ExternalInput")
    t_w0 = nc.dram_tensor("w0", [MEL_D, 3, 256], bf16, kind="ExternalInput")
    t_wam = nc.dram_tensor("wam", [128, 4, 3, 2, 256], bf16, kind="ExternalInput")
    t_wgm = nc.dram_tensor("wgm", [128, 4, 3, 2, 256], f8, kind="ExternalInput")
    t_wph = nc.dram_tensor("wph", [128, 4, 3, 2, 512], f8, kind="ExternalInput")
    t_id1 = nc.dram_tensor("id1", [128, 128], bf16, kind="ExternalInput")
    t_idc4 = nc.dram_tensor("idc4", [128, 128], bf16, kind="ExternalInput")
    t_out = nc.dram_tensor("out", [SPC, T_MEL, 512], bf16, kind="ExternalOutput")

    def bcast(ap, parts):
        return bass.AP(tensor=ap.tensor, offset=ap.offset,
                       ap=[[0, parts]] + list(ap.ap))

    def pbcast(ap):
        # [128, w] -> [128, 2, w] broadcasting over the plane dim
        a = list(ap.ap)
        return bass.AP(tensor=ap.tensor, offset=ap.offset,
                       ap=[list(a[0]), [0, 2], list(a[1])])

    with tile.TileContext(nc) as tc, ExitStack() as ctx:
        wconst = ctx.enter_context(tc.tile_pool(name="wconst", bufs=1))
        state = ctx.enter_context(tc.tile_pool(name="state", bufs=1))
        qpool = ctx.enter_context(tc.tile_pool(name="q", bufs=2))
        mpool = ctx.enter_context(tc.tile_pool(name="mcm", bufs=2))
        sgpool = ctx.enter_context(tc.tile_pool(name="sig", bufs=8))
        epool = ctx.enter_context(tc.tile_pool(name="ets", bufs=4))
        ztpool = ctx.enter_context(tc.tile_pool(name="ztm", bufs=4))
        spool = ctx.enter_context(tc.tile_pool(name="small", bufs=4))
        opool = ctx.enter_context(tc.tile_pool(name="oc", bufs=4))
        ppsum = ctx.enter_context(tc.tile_pool(name="pconv", bufs=7, space="PSUM"))
        tpsum = ctx.enter_context(tc.tile_pool(name="ptp", bufs=1, space="PSUM"))

        # ---- constants (block-split so blk 0 lands first) ----
        w0_t = wconst.tile([MEL_D, 3, 256], bf16, tag="w0")
        nc.scalar.dma_start(out=w0_t[:], in_=t_w0[:])
        id1_t = wconst.tile([128, 128], bf16, tag="id1")
        nc.scalar.dma_start(out=id1_t[:], in_=t_id1[:])
        idc4_t = wconst.tile([128, 128], bf16, tag="idc4")
        nc.scalar.dma_start(out=idc4_t[:], in_=t_idc4[:])
        wam_t = wconst.tile([128, 4, 3, 2, 256], bf16, tag="wam")
        wgm_t = wconst.tile([128, 4, 3, 2, 256], f8, tag="wgm")
        wph_t = wconst.tile([128, 4, 3, 2, 512], f8, tag="wph")

        def load_weights():
            for k in range(3):
                nc.sync.dma_start(out=wam_t[:, 0, k], in_=t_wam[:, 0, k])
            for k in range(3):
                nc.sync.dma_start(out=wgm_t[:, 0, k], in_=t_wgm[:, 0, k])
            nc.sync.dma_start(out=wph_t[:, 0], in_=t_wph[:, 0])
            nc.sync.dma_start(out=wam_t[:, 1:4], in_=t_wam[:, 1:4])
            nc.sync.dma_start(out=wgm_t[:, 1:4], in_=t_wgm[:, 1:4])
            nc.sync.dma_start(out=wph_t[:, 1:4], in_=t_wph[:, 1:4])

        ys = {}
        zs = {}
        mcs = {}
        vbs = {}
        vps = {}

        MCQ = {0: nc.scalar, 1: nc.gpsimd, 2: nc.scalar, 3: nc.gpsimd}

        def load_slot_main(s):
            # mel input in chunk-sized pieces so conv0 starts on piece 0
            mc = mpool.tile([MEL_D, W[s] + 4], bf16, tag=f"mcm{s}", name="mcm",
                            bufs=1)
            prev = 0
            for i, (off, n) in enumerate(mel_chunks[s]):
                hi = W[s] + 4 if i == len(mel_chunks[s]) - 1 else off + n + 3
                MCQ[s].dma_start(out=mc[:, prev:hi], in_=t_mcm[s, :, prev:hi])
                prev = hi
            mcs[s] = mc
            yt = [state.tile([128, 2, YW[s]], bf16, tag=f"y{s}_{i}", name="y")
                  for i in range(2)]
            for i in range(2):
                nc.vector.memset(yt[i][:, :, 0:2], 0.0)
                nc.vector.memset(yt[i][:, :, 2 + W[s]:YW[s]], 0.0)
            ys[s] = yt

        def load_slot_rest(s):
            zt = [state.tile([128, 2, SZ], bf16, tag=f"z{s}_{i}", name="z")
                  for i in range(2)]
            ZQ = {0: nc.scalar, 1: nc.gpsimd, 2: nc.scalar, 3: nc.gpsimd}
            ZQ[s].dma_start(out=zt[0][:],
                            in_=t_zph[s].rearrange("c p w -> p c w"))
            mw = W[s] + 2 - MLO[s]
            vb = wconst.tile([128, mw], bf16, tag=f"vm{s}", name="vm")
            nc.gpsimd.dma_start(out=vb[:], in_=bcast(t_vm[s, MLO[s]:W[s] + 2], 128))
            vbs[s] = vb
            pw = SW[s] + 2 - PLO[s]
            vp = wconst.tile([128, pw], bf16, tag=f"vp{s}", name="vp")
            nc.gpsimd.dma_start(out=vp[:], in_=bcast(t_vph[s, PLO[s]:SW[s] + 2], 128))
            vps[s] = vp
            nc.vector.memset(zt[1][:, :, 0:2], 0.0)
            nc.vector.memset(zt[1][:, :, 2 + SW[s]:SZ], 0.0)
            zs[s] = zt

        def mel_mask(s, dst):
            mw = W[s] + 2 - MLO[s]
            nc.gpsimd.tensor_mul(out=dst[:, :, MLO[s]:W[s] + 2],
                                 in0=dst[:, :, MLO[s]:W[s] + 2],
                                 in1=pbcast(vbs[s][:, 0:mw]))

        def ph_mask(s, dst):
            pw = SW[s] + 2 - PLO[s]
            nc.gpsimd.tensor_mul(out=dst[:, :, PLO[s]:SW[s] + 2],
                                 in0=dst[:, :, PLO[s]:SW[s] + 2],
                                 in1=pbcast(vps[s][:, 0:pw]))

        def conv0(s):
            mc = mcs[s]
            y0 = ys[s][0]
            for (off, n) in mel_chunks[s]:
                for ocb in range(2):
                    pi = ppsum.tile([128, 512], f32, tag="cps", name="cps")
                    for k in range(3):
                        nc.tensor.matmul(pi[:, :n],
                                         w0_t[:, k, 128 * ocb:128 * ocb + 128],
                                         mc[:, off + 1 + k:off + 1 + k + n],
                                         start=(k == 0), stop=(k == 2))
                    nc.scalar.copy(out=y0[:, ocb, off + 2:off + 2 + n],
                                   in_=pi[:, :n])
            mel_mask(s, y0)

        def cast_pieces(dst_q, src_y, chunks, wfull):
            # fp8 cast in chunk-aligned pieces so g-convs start early.
            prev = 0
            for i, (off, n) in enumerate(chunks):
                hi = min(off + n + 3, wfull)
                if hi > prev:
                    nc.vector.tensor_copy(out=dst_q[:, :, prev:hi],
                                          in_=src_y[:, :, prev:hi])
                prev = hi

        ymqs = {}

        def mel_cast(b, s):
            y_old = ys[s][b % 2]
            ymq = qpool.tile([128, 2, max(WQ)], f8, tag="qm", name="ymq")
            cast_pieces(ymq, y_old, mel_chunks[s], W[s] + 4)
            ymqs[s] = ymq

        def mel_glu(b, s):
            y_old = ys[s][b % 2]
            y_new = ys[s][(b + 1) % 2]
            ymq = ymqs[s]
            chunks = mel_chunks[s]
            for oco in range(2):
                def g_phase():
                    pg = {}
                    for (off, n) in chunks:
                        pg[off] = ppsum.tile([128, 512], f32, tag="cps",
                                             name="cps")
                    for k in range(3):
                        wsl = wgm_t[:, b, k, :, 128 * oco:128 * oco + 128]
                        for (off, n) in chunks:
                            nc.tensor.matmul(pg[off][:, :n], wsl,
                                             ymq[:, :, off + 1 + k:off + 1 + k + n],
                                             start=(k == 0), stop=(k == 2),
                                             perf_mode=DR)
                    sigs = {}
                    for (off, n) in chunks:
                        sig = sgpool.tile([128, 512], bf16, tag="sig", name="sig")
                        nc.scalar.activation(out=sig[:, :n], in_=pg[off][:, :n],
                                             func=AF.Sigmoid)
                        sigs[off] = sig
                    return sigs

                def a_phase():
                    pa = {}
                    for (off, n) in chunks:
                        pa[off] = ppsum.tile([128, 512], f32, tag="cps",
                                             name="cps")
                    for k in range(3):
                        for icb in range(2):
                            wsl = wam_t[:, b, k, icb, 128 * oco:128 * oco + 128]
                            st = (k == 0 and icb == 0)
                            sp = (k == 2 and icb == 1)
                            for (off, n) in chunks:
                                nc.tensor.matmul(pa[off][:, :n], wsl,
                                                 y_old[:, icb, off + 1 + k:off + 1 + k + n],
                                                 start=st, stop=sp)
                    return pa

                if b == 0:
                    pa = a_phase()
                    sigs = g_phase()
                else:
                    sigs = g_phase()
                    pa = a_phase()
                for (off, n) in chunks:
                    nc.vector.tensor_mul(out=sigs[off][:, :n], in0=pa[off][:, :n],
                                         in1=sigs[off][:, :n])
                    nc.vector.tensor_add(out=y_new[:, oco, off + 2:off + 2 + n],
                                         in0=sigs[off][:, :n],
                                         in1=y_old[:, oco, off + 2:off + 2 + n])
            if b < 3:
                mel_mask(s, y_new)

        zqs = {}

        def ph_cast(b, s):
            z_old = zs[s][b % 2]
            q = qpool.tile([128, 2, SZQ], f8, tag="qp", name="zq")
            nc.gpsimd.tensor_copy(out=q[:, :, 0:SW[s] + 3],
                                  in_=z_old[:, :, 0:SW[s] + 3])
            zqs[s] = q

        def ph_glu(b, ss):
            zq = zqs
            for oco in range(2):
                pp = {}
                for path in range(2):
                    for s in ss:
                        pp[(s, path)] = ppsum.tile([128, 512], f32, tag="cps",
                                                   name="cps")
                    col0 = 256 * path + 128 * oco
                    for k in range(3):
                        wsl = wph_t[:, b, k, :, col0:col0 + 128]
                        for s in ss:
                            (off, n) = ph_chunks[s][0]
                            nc.tensor.matmul(pp[(s, path)][:, :n], wsl,
                                             zq[s][:, :, off + 1 + k:off + 1 + k + n],
                                             start=(k == 0), stop=(k == 2),
                                             perf_mode=DR)
                for s in ss:
                    n = ph_chunks[s][0][1]
                    z_old = zs[s][b % 2]
                    z_new = zs[s][(b + 1) % 2]
                    sig = sgpool.tile([128, 512], bf16, tag="sig", name="sig")
                    nc.scalar.activation(out=sig[:, :n], in_=pp[(s, 1)][:, :n],
                                         func=AF.Sigmoid)
                    nc.vector.tensor_mul(out=sig[:, :n], in0=pp[(s, 0)][:, :n],
                                         in1=sig[:, :n])
                    nc.vector.tensor_add(out=z_new[:, oco, 2:2 + n],
                                         in0=sig[:, :n],
                                         in1=z_old[:, oco, 2:2 + n])
            if b < 3:
                for s in ss:
                    ph_mask(s, zs[s][(b + 1) % 2])

        def melout_items(s):
            y4 = ys[s][0]

            def tile_job(tt):
                def go():
                    rows = min(128, Tb[s] - 128 * tt)
                    tp = tpsum.tile([128, 256], f32, tag="tp", name="tp")
                    for dcb in range(2):
                        nc.tensor.matmul(tp[:rows, 128 * dcb:128 * dcb + 128],
                                         y4[:, dcb, 2 + 128 * tt:2 + 128 * tt + rows],
                                         idc4_t[:],
                                         start=(dcb == 0), stop=(dcb == 1))
                    om = opool.tile([128, 256], bf16, tag="om", name="om")
                    nc.vector.tensor_copy(out=om[:rows], in_=tp[:rows])
                    nc.scalar.dma_start(
                        out=t_out[s, 128 * tt:128 * tt + rows, 0:256],
                        in_=om[:rows])
                return go

            return [tile_job(tt) for tt in range((Tb[s] + 127) // 128)]

        attn_state = {}

        def attn_pre(s):
            n_sb = NSB[s]
            mv_t = spool.tile([128, n_sb], f32, tag=f"mv{s}", name="mv", bufs=1)
            src = t_mv[s]
            nc.gpsimd.dma_start(out=mv_t[:], in_=bass.AP(
                tensor=src.tensor, offset=src.offset,
                ap=[[1, 128], [128, n_sb]]))
            z4 = zs[s][0]
            zts = []
            biases = []
            for sb in range(n_sb):
                ztm = ztpool.tile([128, 260], bf16, tag=f"zt{s}", name="ztm",
                                  bufs=n_sb)
                tp = tpsum.tile([128, 256], f32, tag="tp", name="tp")
                for dcb in range(2):
                    nc.tensor.matmul(tp[:, 128 * dcb:128 * dcb + 128],
                                     z4[:, dcb, 2 + 128 * sb:130 + 128 * sb],
                                     id1_t[:],
                                     start=(dcb == 0), stop=(dcb == 1))
                nc.vector.tensor_copy(out=ztm[:, 0:256], in_=tp[:])
                nc.vector.memset(ztm[:, 256:260], 4.0)
                sq = opool.tile([128, 256], f32, tag="sq", name="sq", bufs=2)
                nc.gpsimd.tensor_mul(out=sq[:], in0=ztm[:, 0:256],
                                     in1=ztm[:, 0:256])
                ph2 = spool.tile([128, 1], f32, tag="ph2", name="ph2")
                nc.vector.tensor_reduce(out=ph2[:], in_=sq[:], axis=AX.X,
                                        op=ALU.add)
                bias_sb = spool.tile([128, 1], f32, tag=f"bias{s}", name="bias",
                                     bufs=n_sb)
                nc.gpsimd.tensor_scalar(out=bias_sb[:], in0=ph2[:],
                                        scalar1=-C8, scalar2=mv_t[:, sb:sb + 1],
                                        op0=ALU.mult, op1=ALU.add)
                zts.append(ztm)
                biases.append(bias_sb)
            attn_state[s] = (zts, biases, [None] * n_sb)

        def dots_exp_item(s, sb, off, n):
            zts, biases, ets = attn_state[s]
            if ets[sb] is None:
                ets[sb] = epool.tile([128, Tb[s]], bf16, tag=f"et{s}",
                                     name="et", bufs=NSB[s])
            z4 = zs[s][0]
            y4 = ys[s][0]
            dp = ppsum.tile([128, 512], f32, tag="cps", name="dps")
            for dcb in range(2):
                nc.tensor.matmul(dp[:, :n],
                                 z4[:, dcb, 2 + 128 * sb:130 + 128 * sb],
                                 y4[:, dcb, 2 + off:2 + off + n],
                                 start=(dcb == 0), stop=(dcb == 1))
            nc.scalar.activation(out=ets[sb][:, off:off + n], in_=dp[:, :n],
                                 func=AF.Exp, bias=biases[sb], scale=2 * C8)

        def dots_items(s):
            def job(sb, off, n):
                return lambda: dots_exp_item(s, sb, off, n)
            return [job(sb, off, n)
                    for (off, n) in dot_chunks[s]
                    for sb in range(NSB[s])]

        def ctx_items(s):
            n_sb = NSB[s]

            def tile_job(tt):
                def go():
                    zts, biases, ets = attn_state[s]
                    rows = min(128, Tb[s] - 128 * tt)
                    cp = ppsum.tile([128, 512], f32, tag="cps", name="cxs")
                    for sb in range(n_sb):
                        nc.tensor.matmul(cp[:rows, 0:260],
                                         ets[sb][:, 128 * tt:128 * tt + rows],
                                         zts[sb][:],
                                         start=(sb == 0), stop=(sb == n_sb - 1))
                    rc = spool.tile([128, 1], f32, tag="rc", name="rc")
                    nc.vector.reciprocal(out=rc[:rows], in_=cp[:rows, 256:257])
                    oc = opool.tile([128, 256], bf16, tag="oc", name="oc")
                    nc.vector.tensor_scalar_mul(out=oc[:rows],
                                                in0=cp[:rows, 0:256],
                                                scalar1=rc[:rows])
                    nc.sync.dma_start(
                        out=t_out[s, 128 * tt:128 * tt + rows, 256:512],
                        in_=oc[:rows])
                return go

            return [tile_job(tt) for tt in range((Tb[s] + 127) // 128)]

        # ---- emission ----
        load_slot_main(0)
        load_slot_main(1)
        load_weights()
        load_slot_rest(0)
        load_slot_rest(1)
        conv0(0)
        conv0(1)

        def rr(*worklists):
            worklists = [list(w) for w in worklists]
            out = []
            i = 0
            while any(worklists):
                wl = worklists[i % len(worklists)]
                if wl:
                    out.append(wl.pop(0))
                i += 1
            return out

        def run(jobs):
            for j in jobs:
                j()

        def take(work, n):
            for _ in range(n):
                if work:
                    work.pop(0)()

        # phase 1: big slots' full GLU stack; casts emitted right after the
        # producing slot's pointwise so the next block never waits on them
        mel_cast(0, 0)
        mel_cast(0, 1)
        ph_cast(0, 0)
        ph_cast(0, 1)
        for b in range(4):
            if b == 1:
                load_slot_main(2)
                load_slot_main(3)
                load_slot_rest(2)
                load_slot_rest(3)
            mel_glu(b, 0)
            if b < 3:
                mel_cast(b + 1, 0)
            mel_glu(b, 1)
            if b < 3:
                mel_cast(b + 1, 1)
            if b == 3:
                conv0(2)
                conv0(3)
            ph_glu(b, (0, 1))
            if b < 3:
                ph_cast(b + 1, 0)
                ph_cast(b + 1, 1)
        # bridge 1: mel_out head start fills the wait for the last ph adds,
        # then dense dots+exp for (0,1)
        mo0 = melout_items(0)
        mo1 = melout_items(1)
        run(mo0[:4] + mo1[:3])
        attn_pre(0)
        attn_pre(1)
        run(rr(dots_items(0), dots_items(1), mo0[4:], mo1[3:]))
        run(ctx_items(0))
        # phase 2: small slots' GLU with scalar-free wedges (ctx of slot 1)
        work = list(ctx_items(1))
        nw = len(work)
        mel_cast(0, 2)
        mel_cast(0, 3)
        ph_cast(0, 2)
        ph_cast(0, 3)
        for b in range(4):
            mel_glu(b, 2)
            if b < 3:
                mel_cast(b + 1, 2)
            take(work, nw // 12)
            mel_glu(b, 3)
            if b < 3:
                mel_cast(b + 1, 3)
            take(work, nw // 12)
            ph_glu(b, (2, 3))
            if b < 3:
                ph_cast(b + 1, 2)
                ph_cast(b + 1, 3)
            take(work, nw // 12)
        run(work)
        # phase 3: small slots' attention tail
        attn_pre(2)
        attn_pre(3)
        run(rr(dots_items(2), dots_items(3), melout_items(2), melout_items(3)))
        run(rr(ctx_items(2), ctx_items(3)))

    if not nc.is_finalized():
        nc.finalize()
    return nc


def _get_program(S_pad, L, SL, LO, LOPH):
    key = (S_pad, L, SL, LO, LOPH)
    if key not in _prog_cache:
        _prog_cache[key] = _build_program(S_pad, L, SL, LO, LOPH)
    return _prog_cache[key]


LAST_RESULTS = None


def _install_ntff_hook():
    """Provide antenv.axon_hooks (missing in this image) so trace=True works."""
    import sys
    import types
    import ctypes
    import contextlib
    if "antenv.axon_hooks" in sys.modules:
        return
    try:
        import antenv
    except ImportError:
        return
    mod = types.ModuleType("antenv.axon_hooks")
    state = {}
    mod.set_axon_ntff_profile_hook = lambda h: state.__setitem__("h", h)
    mod.get_axon_ntff_profile_hook = lambda: state.get("h")
    sys.modules["antenv.axon_hooks"] = mod
    antenv.axon_hooks = mod
    so_path = "/opt/axon/libaxon_pjrt.so"
    if not os.path.exists(so_path):
        return
    lib = ctypes.CDLL(so_path)
    if not hasattr(lib, "axon_start_nrt_profile"):
        return
    lib.axon_start_nrt_profile.argtypes = [ctypes.POINTER(ctypes.c_int64),
                                           ctypes.c_size_t]
    lib.axon_start_nrt_profile.restype = ctypes.c_int64
    lib.axon_stop_nrt_profile.argtypes = [ctypes.c_char_p]
    lib.axon_stop_nrt_profile.restype = ctypes.c_int64

    @contextlib.contextmanager
    def _hook(output_dir, device_ids):
        import jax
        jax.devices()
        if device_ids:
            ids = (ctypes.c_int64 * len(device_ids))(*device_ids)
            rc = lib.axon_start_nrt_profile(ids, len(device_ids))
        else:
            rc = lib.axon_start_nrt_profile(None, 0)
        if rc != 0:
            raise RuntimeError(f"axon_start_nrt_profile rc={rc}")
        try:
            yield
        finally:
            n = lib.axon_stop_nrt_profile(str(output_dir).encode())
            print(f"ntff profile: {n} file(s) -> {output_dir}")

    mod.set_axon_ntff_profile_hook(_hook)


def kernel(mels, phonemes, mel_lens, phoneme_lens, embedding,
           mel_conv_w, mel_conv_b, ph_w, ph_b, mel_w, mel_b):
    global LAST_RESULTS
    from concourse.bass_utils import run_bass_kernel_spmd

    mels = np.asarray(mels)
    assert mels.shape == (B, T_MEL, MEL_D), mels.shape
    max_pl = int(np.max(np.asarray(phoneme_lens)))
    S_pad = 512 if max_pl <= 511 else 640

    in_maps, perm, L, SL, LO, LOPH = _host_prep(
        np.asarray(mels), np.asarray(phonemes), np.asarray(mel_lens),
        np.asarray(phoneme_lens), np.asarray(embedding),
        np.asarray(mel_conv_w), np.asarray(mel_conv_b),
        np.asarray(ph_w), np.asarray(ph_b),
        np.asarray(mel_w), np.asarray(mel_b), S_pad)

    nc = _get_program(S_pad, L, SL, LO, LOPH)
    trace = bool(int(os.environ.get("KERNEL_TRACE", "0")))
    if trace:
        _install_ntff_hook()
    res = run_bass_kernel_spmd(nc, in_maps, core_ids=list(range(N_CORES)),
                               trace=trace,
                               tmpdir=os.environ.get("KERNEL_TRACE_DIR"))
    LAST_RESULTS = res
    out = np.empty((B, T_MEL, 512), np.float32)
    Tb = [min(T_MEL, _ceil(L[j] + 2, 128)) for j in range(SPC)]
    for c in range(N_CORES):
        for j in range(SPC):
            bi = int(perm[8 * j + c])
            out[bi] = np.asarray(res.results[c]["out"][j], dtype=np.float32)
            if Tb[j] < T_MEL:
                out[bi, Tb[j]:, :256] = 0.0
                out[bi, Tb[j]:, 256:] = out[bi, L[j] + 1, 256:]
    return out


# revision 19
# speedup vs baseline: 1.2187x; 1.2187x over previous
"""Trainium2 Bass kernel for nn_AligningModel (mel/phoneme GLU encoders + soft attention).

Strategy (v2):
  - Data-parallel over batch: 32 samples -> 8 cores x 4 slots, length-sorted so
    each slot's compile-time bound is tight (slot j holds sorted ranks 8j..8j+7).
  - bf16 datapath everywhere; fp8e4m3 DoubleRow matmuls (256-contract/pass) for
    the mel GLU gate path and the entire phoneme encoder (error-tolerant paths,
    validated via numpy: ~7e-3 final rel err vs 2e-2 gate). Mel a-path stays bf16.
  - Ping-pong y tiles per GLU block: conv reads the old tile, the residual add
    writes the new one -> no masked input copy.  Masking is a narrow post-mask
    on cols [min_len_in_group+1, W+2) only (cols below are valid in all cores).
  - All conv weights SBUF-resident (bf16/fp8, ~3.5MB), DMA'd once per-block so
    block 0's weights land before the first GLU needs them.
  - Weight reuse: mel convs stream chunk-pairs per stationary load; phoneme
    convs stream both slots of a pair per load (hides 256-col DoubleRow LDW).
  - Scale folding: sqrt(0.5)^b folded into g-path conv weights; softmax uses
    logits = 2*C^8*dots - C^8*ph_sq (mel_sq dropped: softmax-invariant),
    no max-subtraction, phoneme -1e9 mask folded into per-partition exp bias.
  - Z (softmax denominator) via ones-columns appended to time-major ph tiles.
  - Attention emitted interleaved across slot pairs (PE dots ping-pong with
    scalar exp) with mel_out identity-matmuls as PE filler to stay HAM-warm.
"""

import os
import numpy as np
import ml_dtypes

B = 32
N_CORES = 8
SPC = 4           # samples (slots) per core
T_MEL = 2000
MEL_D = 80
D = 256
C = float(np.sqrt(0.5))
C4 = 0.25         # C**4 exact
C8 = 0.0625       # C**8 exact

BF = ml_dtypes.bfloat16
F8 = ml_dtypes.float8_e4m3

_prog_cache = {}


def _chunks(total, cap=512):
    out = []
    off = 0
    while off < total:
        w = min(cap, total - off)
        out.append((off, w))
        off += w
    return out


def _pairs(lst):
    return [lst[i:i + 2] for i in range(0, len(lst), 2)]


def _ceil(x, m):
    return -(-x // m) * m


def _host_prep(mels, phonemes, mel_lens, phoneme_lens, embedding,
               mel_conv_w, mel_conv_b, ph_w, ph_b, mel_w, mel_b, S_pad):
    """Build the per-core input maps (numpy only). Returns (in_maps, perm,
    L, SL, LO, LOPH) where perm[8*j + c] = original sample index in core c
    slot j."""
    f32 = np.float32
    SP2 = S_pad + 2

    order = np.argsort(-np.asarray(mel_lens), kind="stable")
    perm = np.asarray(order)
    L = tuple(int(mel_lens[perm[8 * j]]) for j in range(SPC))
    SL = tuple(int(max(phoneme_lens[perm[8 * j + c]] for c in range(8)))
               for j in range(SPC))
    LO = tuple(int(min(mel_lens[perm[8 * j + c]] for c in range(8)))
               for j in range(SPC))
    LOPH = tuple(int(min(phoneme_lens[perm[8 * j + c]] for c in range(8)))
                 for j in range(SPC))

    assert not np.any(mel_conv_b) and not np.any(mel_b) and not np.any(ph_b), \
        "nonzero conv biases not supported by this kernel variant"

    # conv0 weights: [i, k, o] bf16
    w0h = np.ascontiguousarray(
        np.transpose(mel_conv_w.astype(f32), (1, 2, 0))).astype(BF)

    scale = (C ** np.arange(4, dtype=np.float64)).astype(f32)

    def enc_layout(w4):
        # w4: [4, O, 256, 3] -> [128(ci), 4(b), 3(k), 2(icb), O]
        t = np.transpose(w4, (2, 0, 3, 1))          # [i, b, k, o]
        t = t.reshape(2, 128, 4, 3, w4.shape[1])    # [icb, ci, b, k, o]
        return np.ascontiguousarray(np.transpose(t, (1, 2, 3, 0, 4)))

    wa = enc_layout(mel_w[:, :256].astype(f32)).astype(BF)
    wg = enc_layout(mel_w[:, 256:].astype(f32)
                    * scale[:, None, None, None]).astype(F8)
    wpf = ph_w.astype(f32).copy()
    wpf[:, 256:] *= scale[:, None, None, None]
    wp = enc_layout(wpf).astype(F8)

    id1 = np.eye(128, dtype=f32).astype(BF)
    idc4 = (C4 * np.eye(128, dtype=f32)).astype(BF)

    shared = {"w0": w0h, "wam": wa, "wgm": wg, "wph": wp,
              "id1": id1, "idc4": idc4}

    ar = np.arange(T_MEL)
    ars = np.arange(S_pad)
    in_maps = []
    for c in range(N_CORES):
        idx = [int(perm[8 * j + c]) for j in range(SPC)]
        m = dict(shared)
        mcm = np.zeros((SPC, MEL_D, T_MEL + 4), BF)
        vm = np.zeros((SPC, T_MEL + 4), BF)
        zph = np.zeros((SPC, 2, 128, S_pad + 4), BF)
        vph = np.zeros((SPC, S_pad + 4), BF)
        mv = np.full((SPC, S_pad), -1e9, f32)
        for j, b in enumerate(idx):
            mcm[j, :, 2:T_MEL + 2] = np.asarray(mels[b], f32).T.astype(BF)
            vm[j, 2:T_MEL + 2] = (ar < int(mel_lens[b])).astype(BF)
            pl = int(phoneme_lens[b])
            ph_pad = np.concatenate([[0], np.asarray(phonemes[b], np.int64)])[:S_pad]
            e = embedding[ph_pad].astype(f32)
            valid = (ars[:len(e)] <= pl)
            e[~valid] = 0.0
            zph[j, :, :, 2:2 + len(e)] = e.T.reshape(2, 128, len(e)).astype(BF)
            vph[j, 2:2 + len(e)] = valid.astype(BF)
            mv[j, :len(e)][valid] = 0.0
        m["mels_cm"] = mcm
        m["valid_mel"] = vm
        m["zph0"] = zph
        m["valid_ph"] = vph
        m["mvec"] = mv
        in_maps.append(m)
    return in_maps, perm, L, SL, LO, LOPH


def _build_program(S_pad, L, SL, LO, LOPH):
    from contextlib import ExitStack
    import concourse.bass as bass
    import concourse.bacc as bacc
    import concourse.tile as tile
    from concourse import mybir

    f32 = mybir.dt.float32
    bf16 = mybir.dt.bfloat16
    f8 = mybir.dt.float8e4
    AF = mybir.ActivationFunctionType
    ALU = mybir.AluOpType
    AX = mybir.AxisListType
    DR = mybir.MatmulPerfMode.DoubleRow
    SZ = S_pad + 4                 # z tile width (data at col offset 2)
    SZQ = _ceil(SZ, 16)

    # per-slot compile-time bounds
    W = [min(T_MEL, _ceil(L[j] + 2, 4)) for j in range(SPC)]       # mel conv cols
    Tb = [min(T_MEL, _ceil(L[j] + 2, 128)) for j in range(SPC)]    # attn rows
    SW = [min(S_pad, _ceil(SL[j] + 2, 4)) for j in range(SPC)]     # ph conv cols
    NSB = [min(S_pad // 128, _ceil(SL[j] + 2, 128) // 128) for j in range(SPC)]
    WQ = [_ceil(W[j] + 4, 16) for j in range(SPC)]
    YW = [Tb[j] + 4 for j in range(SPC)]
    mel_chunks = [_chunks(W[j]) for j in range(SPC)]
    ph_chunks = [_chunks(SW[j]) for j in range(SPC)]
    dot_chunks = [_chunks(Tb[j]) for j in range(SPC)]
    # narrow post-mask regions (first possibly-invalid data col .. end of data)
    MLO = [min(LO[j] + 2, W[j] + 2) for j in range(SPC)]
    PLO = [min(LOPH[j] + 3, SW[j] + 2) for j in range(SPC)]

    nc = bacc.Bacc()
    t_mcm = nc.dram_tensor("mels_cm", [SPC, MEL_D, T_MEL + 4], bf16, kind="ExternalInput")
    t_vm = nc.dram_tensor("valid_mel", [SPC, T_MEL + 4], bf16, kind="ExternalInput")
    t_zph = nc.dram_tensor("zph0", [SPC, 2, 128, SZ], bf16, kind="ExternalInput")
    t_vph = nc.dram_tensor("valid_ph", [SPC, SZ], bf16, kind="ExternalInput")
    t_mv = nc.dram_tensor("mvec", [SPC, S_pad], f32, kind="ExternalInput")
    t_w0 = nc.dram_tensor("w0", [MEL_D, 3, 256], bf16, kind="ExternalInput")
    t_wam = nc.dram_tensor("wam", [128, 4, 3, 2, 256], bf16, kind="ExternalInput")
    t_wgm = nc.dram_tensor("wgm", [128, 4, 3, 2, 256], f8, kind="ExternalInput")
    t_wph = nc.dram_tensor("wph", [128, 4, 3, 2, 512], f8, kind="ExternalInput")
    t_id1 = nc.dram_tensor("id1", [128, 128], bf16, kind="ExternalInput")
    t_idc4 = nc.dram_tensor("idc4", [128, 128], bf16, kind="ExternalInput")
    t_out = nc.dram_tensor("out", [SPC, T_MEL, 512], bf16, kind="ExternalOutput")

    def bcast(ap, parts):
        return bass.AP(tensor=ap.tensor, offset=ap.offset,
                       ap=[[0, parts]] + list(ap.ap))

    def pbcast(ap):
        # [128, w] -> [128, 2, w] broadcasting over the plane dim
        a = list(ap.ap)
        return bass.AP(tensor=ap.tensor, offset=ap.offset,
                       ap=[list(a[0]), [0, 2], list(a[1])])

    with tile.TileContext(nc) as tc, ExitStack() as ctx:
        wconst = ctx.enter_context(tc.tile_pool(name="wconst", bufs=1))
        state = ctx.enter_context(tc.tile_pool(name="state", bufs=1))
        qpool = ctx.enter_context(tc.tile_pool(name="q", bufs=2))
        mpool = ctx.enter_context(tc.tile_pool(name="mcm", bufs=2))
        sgpool = ctx.enter_context(tc.tile_pool(name="sig", bufs=8))
        epool = ctx.enter_context(tc.tile_pool(name="ets", bufs=4))
        ztpool = ctx.enter_context(tc.tile_pool(name="ztm", bufs=4))
        spool = ctx.enter_context(tc.tile_pool(name="small", bufs=4))
        opool = ctx.enter_context(tc.tile_pool(name="oc", bufs=4))
        ppsum = ctx.enter_context(tc.tile_pool(name="pconv", bufs=6, space="PSUM"))
        tpsum = ctx.enter_context(tc.tile_pool(name="ptp", bufs=2, space="PSUM"))

        # ---- constants (block-split so blk 0 lands first) ----
        w0_t = wconst.tile([MEL_D, 3, 256], bf16, tag="w0")
        nc.scalar.dma_start(out=w0_t[:], in_=t_w0[:])
        id1_t = wconst.tile([128, 128], bf16, tag="id1")
        nc.scalar.dma_start(out=id1_t[:], in_=t_id1[:])
        idc4_t = wconst.tile([128, 128], bf16, tag="idc4")
        nc.scalar.dma_start(out=idc4_t[:], in_=t_idc4[:])
        wam_t = wconst.tile([128, 4, 3, 2, 256], bf16, tag="wam")
        wgm_t = wconst.tile([128, 4, 3, 2, 256], f8, tag="wgm")
        wph_t = wconst.tile([128, 4, 3, 2, 512], f8, tag="wph")

        def load_weights():
            for k in range(3):
                nc.sync.dma_start(out=wam_t[:, 0, k], in_=t_wam[:, 0, k])
            for k in range(3):
                nc.sync.dma_start(out=wgm_t[:, 0, k], in_=t_wgm[:, 0, k])
            nc.sync.dma_start(out=wph_t[:, 0], in_=t_wph[:, 0])
            nc.sync.dma_start(out=wam_t[:, 1:4], in_=t_wam[:, 1:4])
            nc.sync.dma_start(out=wgm_t[:, 1:4], in_=t_wgm[:, 1:4])
            nc.sync.dma_start(out=wph_t[:, 1:4], in_=t_wph[:, 1:4])

        ys = {}
        zs = {}
        mcs = {}
        vbs = {}
        vps = {}

        MCQ = {0: nc.scalar, 1: nc.gpsimd, 2: nc.scalar, 3: nc.gpsimd}

        def load_slot_main(s):
            # mel input in chunk-sized pieces so conv0 starts on piece 0
            mc = mpool.tile([MEL_D, W[s] + 4], bf16, tag=f"mcm{s}", name="mcm",
                            bufs=1)
            prev = 0
            for i, (off, n) in enumerate(mel_chunks[s]):
                hi = W[s] + 4 if i == len(mel_chunks[s]) - 1 else off + n + 3
                MCQ[s].dma_start(out=mc[:, prev:hi], in_=t_mcm[s, :, prev:hi])
                prev = hi
            mcs[s] = mc
            yt = [state.tile([128, 2, YW[s]], bf16, tag=f"y{s}_{i}", name="y")
                  for i in range(2)]
            for i in range(2):
                nc.vector.memset(yt[i][:, :, 0:2], 0.0)
                nc.vector.memset(yt[i][:, :, 2 + W[s]:YW[s]], 0.0)
            ys[s] = yt

        def load_slot_rest(s):
            zt = [state.tile([128, 2, SZ], bf16, tag=f"z{s}_{i}", name="z")
                  for i in range(2)]
            ZQ = {0: nc.scalar, 1: nc.gpsimd, 2: nc.scalar, 3: nc.gpsimd}
            ZQ[s].dma_start(out=zt[0][:],
                            in_=t_zph[s].rearrange("c p w -> p c w"))
            mw = W[s] + 2 - MLO[s]
            vb = wconst.tile([128, mw], bf16, tag=f"vm{s}", name="vm")
            nc.gpsimd.dma_start(out=vb[:], in_=bcast(t_vm[s, MLO[s]:W[s] + 2], 128))
            vbs[s] = vb
            pw = SW[s] + 2 - PLO[s]
            vp = wconst.tile([128, pw], bf16, tag=f"vp{s}", name="vp")
            nc.gpsimd.dma_start(out=vp[:], in_=bcast(t_vph[s, PLO[s]:SW[s] + 2], 128))
            vps[s] = vp
            nc.vector.memset(zt[1][:, :, 0:2], 0.0)
            nc.vector.memset(zt[1][:, :, 2 + SW[s]:SZ], 0.0)
            zs[s] = zt

        def mel_mask(s, dst):
            mw = W[s] + 2 - MLO[s]
            nc.gpsimd.tensor_mul(out=dst[:, :, MLO[s]:W[s] + 2],
                                 in0=dst[:, :, MLO[s]:W[s] + 2],
                                 in1=pbcast(vbs[s][:, 0:mw]))

        def ph_mask(s, dst):
            pw = SW[s] + 2 - PLO[s]
            nc.gpsimd.tensor_mul(out=dst[:, :, PLO[s]:SW[s] + 2],
                                 in0=dst[:, :, PLO[s]:SW[s] + 2],
                                 in1=pbcast(vps[s][:, 0:pw]))

        def conv0(s):
            mc = mcs[s]
            y0 = ys[s][0]
            for (off, n) in mel_chunks[s]:
                for ocb in range(2):
                    pi = ppsum.tile([128, 512], f32, tag="cps", name="cps")
                    for k in range(3):
                        nc.tensor.matmul(pi[:, :n],
                                         w0_t[:, k, 128 * ocb:128 * ocb + 128],
                                         mc[:, off + 1 + k:off + 1 + k + n],
                                         start=(k == 0), stop=(k == 2))
                    nc.scalar.copy(out=y0[:, ocb, off + 2:off + 2 + n],
                                   in_=pi[:, :n])
            mel_mask(s, y0)

        def cast_pieces(dst_q, src_y, chunks, wfull):
            # fp8 cast in chunk-aligned pieces so g-convs start early.
            prev = 0
            for i, (off, n) in enumerate(chunks):
                hi = min(off + n + 3, wfull)
                if hi > prev:
                    nc.vector.tensor_copy(out=dst_q[:, :, prev:hi],
                                          in_=src_y[:, :, prev:hi])
                prev = hi

        ymqs = {}

        def mel_cast(b, s):
            y_old = ys[s][b % 2]
            ymq = qpool.tile([128, 2, max(WQ)], f8, tag="qm", name="ymq")
            cast_pieces(ymq, y_old, mel_chunks[s], W[s] + 4)
            ymqs[s] = ymq

        def mel_glu(b, s):
            y_old = ys[s][b % 2]
            y_new = ys[s][(b + 1) % 2]
            ymq = ymqs[s]
            chunks = mel_chunks[s]
            for oco in range(2):
                def g_phase():
                    pg = {}
                    for (off, n) in chunks:
                        pg[off] = ppsum.tile([128, 512], f32, tag="cps",
                                             name="cps")
                    for k in range(3):
                        wsl = wgm_t[:, b, k, :, 128 * oco:128 * oco + 128]
                        for (off, n) in chunks:
                            nc.tensor.matmul(pg[off][:, :n], wsl,
                                             ymq[:, :, off + 1 + k:off + 1 + k + n],
                                             start=(k == 0), stop=(k == 2),
                                             perf_mode=DR)
                    sigs = {}
                    for (off, n) in chunks:
                        sig = sgpool.tile([128, 512], bf16, tag="sig", name="sig")
                        nc.scalar.activation(out=sig[:, :n], in_=pg[off][:, :n],
                                             func=AF.Sigmoid)
                        sigs[off] = sig
                    return sigs

                def a_phase():
                    pa = {}
                    for (off, n) in chunks:
                        pa[off] = ppsum.tile([128, 512], f32, tag="cps",
                                             name="cps")
                    for k in range(3):
                        for icb in range(2):
                            wsl = wam_t[:, b, k, icb, 128 * oco:128 * oco + 128]
                            st = (k == 0 and icb == 0)
                            sp = (k == 2 and icb == 1)
                            for (off, n) in chunks:
                                nc.tensor.matmul(pa[off][:, :n], wsl,
                                                 y_old[:, icb, off + 1 + k:off + 1 + k + n],
                                                 start=st, stop=sp)
                    return pa

                if b == 0:
                    pa = a_phase()
                    sigs = g_phase()
                else:
                    sigs = g_phase()
                    pa = a_phase()
                for (off, n) in chunks:
                    nc.vector.tensor_mul(out=sigs[off][:, :n], in0=pa[off][:, :n],
                                         in1=sigs[off][:, :n])
                    nc.vector.tensor_add(out=y_new[:, oco, off + 2:off + 2 + n],
                                         in0=sigs[off][:, :n],
                                         in1=y_old[:, oco, off + 2:off + 2 + n])
            if b < 3:
                mel_mask(s, y_new)

        zqs = {}

        def ph_cast(b, s):
            z_old = zs[s][b % 2]
            q = qpool.tile([128, 2, SZQ], f8, tag="qp", name="zq")
            nc.gpsimd.tensor_copy(out=q[:, :, 0:SW[s] + 3],
                                  in_=z_old[:, :, 0:SW[s] + 3])
            zqs[s] = q

        def ph_glu(b, ss):
            zq = zqs
            for oco in range(2):
                pp = {}
                for path in range(2):
                    for s in ss:
                        pp[(s, path)] = ppsum.tile([128, 512], f32, tag="cps",
                                                   name="cps")
                    col0 = 256 * path + 128 * oco
                    for k in range(3):
                        wsl = wph_t[:, b, k, :, col0:col0 + 128]
                        for s in ss:
                            (off, n) = ph_chunks[s][0]
                            nc.tensor.matmul(pp[(s, path)][:, :n], wsl,
                                             zq[s][:, :, off + 1 + k:off + 1 + k + n],
                                             start=(k == 0), stop=(k == 2),
                                             perf_mode=DR)
                for s in ss:
                    n = ph_chunks[s][0][1]
                    z_old = zs[s][b % 2]
                    z_new = zs[s][(b + 1) % 2]
                    sig = sgpool.tile([128, 512], bf16, tag="sig", name="sig")
                    nc.scalar.activation(out=sig[:, :n], in_=pp[(s, 1)][:, :n],
                                         func=AF.Sigmoid)
                    nc.vector.tensor_mul(out=sig[:, :n], in0=pp[(s, 0)][:, :n],
                                         in1=sig[:, :n])
                    nc.vector.tensor_add(out=z_new[:, oco, 2:2 + n],
                                         in0=sig[:, :n],
                                         in1=z_old[:, oco, 2:2 + n])
            if b < 3:
                for s in ss:
                    ph_mask(s, zs[s][(b + 1) % 2])

        def melout_items(s):
            y4 = ys[s][0]

            def tile_job(tt):
                def go():
                    rows = min(128, Tb[s] - 128 * tt)
                    tp = tpsum.tile([128, 256], f32, tag="tp", name="tp")
                    for dcb in range(2):
                        nc.tensor.matmul(tp[:rows, 128 * dcb:128 * dcb + 128],
                                         y4[:, dcb, 2 + 128 * tt:2 + 128 * tt + rows],
                                         idc4_t[:],
                                         start=(dcb == 0), stop=(dcb == 1))
                    om = opool.tile([128, 256], bf16, tag="om", name="om")
                    nc.vector.tensor_copy(out=om[:rows], in_=tp[:rows])
                    nc.scalar.dma_start(
                        out=t_out[s, 128 * tt:128 * tt + rows, 0:256],
                        in_=om[:rows])
                return go

            return [tile_job(tt) for tt in range((Tb[s] + 127) // 128)]

        attn_state = {}

        def attn_pre(s):
            n_sb = NSB[s]
            mv_t = spool.tile([128, n_sb], f32, tag=f"mv{s}", name="mv", bufs=1)
            src = t_mv[s]
            nc.gpsimd.dma_start(out=mv_t[:], in_=bass.AP(
                tensor=src.tensor, offset=src.offset,
                ap=[[1, 128], [128, n_sb]]))
            z4 = zs[s][0]
            zts = []
            biases = []
            for sb in range(n_sb):
                ztm = ztpool.tile([128, 260], bf16, tag=f"zt{s}", name="ztm",
                                  bufs=n_sb)
                tp = tpsum.tile([128, 256], f32, tag="tp", name="tp")
                for dcb in range(2):
                    nc.tensor.matmul(tp[:, 128 * dcb:128 * dcb + 128],
                                     z4[:, dcb, 2 + 128 * sb:130 + 128 * sb],
                                     id1_t[:],
                                     start=(dcb == 0), stop=(dcb == 1))
                nc.vector.tensor_copy(out=ztm[:, 0:256], in_=tp[:])
                nc.vector.memset(ztm[:, 256:260], 4.0)
                sq = opool.tile([128, 256], f32, tag="sq", name="sq", bufs=2)
                nc.gpsimd.tensor_mul(out=sq[:], in0=ztm[:, 0:256],
                                     in1=ztm[:, 0:256])
                ph2 = spool.tile([128, 1], f32, tag="ph2", name="ph2")
                nc.vector.tensor_reduce(out=ph2[:], in_=sq[:], axis=AX.X,
                                        op=ALU.add)
                bias_sb = spool.tile([128, 1], f32, tag=f"bias{s}", name="bias",
                                     bufs=n_sb)
                nc.gpsimd.tensor_scalar(out=bias_sb[:], in0=ph2[:],
                                        scalar1=-C8, scalar2=mv_t[:, sb:sb + 1],
                                        op0=ALU.mult, op1=ALU.add)
                zts.append(ztm)
                biases.append(bias_sb)
            attn_state[s] = (zts, biases, [None] * n_sb)

        def dots_exp_item(s, sb, off, n):
            zts, biases, ets = attn_state[s]
            if ets[sb] is None:
                ets[sb] = epool.tile([128, Tb[s]], bf16, tag=f"et{s}",
                                     name="et", bufs=NSB[s])
            z4 = zs[s][0]
            y4 = ys[s][0]
            dp = ppsum.tile([128, 512], f32, tag="cps", name="dps")
            for dcb in range(2):
                nc.tensor.matmul(dp[:, :n],
                                 z4[:, dcb, 2 + 128 * sb:130 + 128 * sb],
                                 y4[:, dcb, 2 + off:2 + off + n],
                                 start=(dcb == 0), stop=(dcb == 1))
            nc.scalar.activation(out=ets[sb][:, off:off + n], in_=dp[:, :n],
                                 func=AF.Exp, bias=biases[sb], scale=2 * C8)

        def dots_items(s):
            def job(sb, off, n):
                return lambda: dots_exp_item(s, sb, off, n)
            return [job(sb, off, n)
                    for (off, n) in dot_chunks[s]
                    for sb in range(NSB[s])]

        def ctx_items(s):
            n_sb = NSB[s]

            def tile_job(tt):
                def go():
                    zts, biases, ets = attn_state[s]
                    rows = min(128, Tb[s] - 128 * tt)
                    cp = ppsum.tile([128, 512], f32, tag="cps", name="cxs")
                    for sb in range(n_sb):
                        nc.tensor.matmul(cp[:rows, 0:260],
                                         ets[sb][:, 128 * tt:128 * tt + rows],
                                         zts[sb][:],
                                         start=(sb == 0), stop=(sb == n_sb - 1))
                    rc = spool.tile([128, 1], f32, tag="rc", name="rc")
                    nc.vector.reciprocal(out=rc[:rows], in_=cp[:rows, 256:257])
                    oc = opool.tile([128, 256], bf16, tag="oc", name="oc")
                    nc.vector.tensor_scalar_mul(out=oc[:rows],
                                                in0=cp[:rows, 0:256],
                                                scalar1=rc[:rows])
                    nc.sync.dma_start(
                        out=t_out[s, 128 * tt:128 * tt + rows, 256:512],
                        in_=oc[:rows])
                return go

            return [tile_job(tt) for tt in range((Tb[s] + 127) // 128)]

        # ---- emission ----
        load_slot_main(0)
        load_slot_main(1)
        load_weights()
        load_slot_rest(0)
        load_slot_rest(1)
        conv0(0)
        conv0(1)

        def rr(*worklists):
            worklists = [list(w) for w in worklists]
            out = []
            i = 0
            while any(worklists):
                wl = worklists[i % len(worklists)]
                if wl:
                    out.append(wl.pop(0))
                i += 1
            return out

        def run(jobs):
            for j in jobs:
                j()

        def take(work, n):
            for _ in range(n):
                if work:
                    work.pop(0)()

        # phase 1: big slots' full GLU stack; casts emitted right after the
        # producing slot's pointwise so the next block never waits on them
        mel_cast(0, 0)
        mel_cast(0, 1)
        ph_cast(0, 0)
        ph_cast(0, 1)
        for b in range(4):
            if b == 1:
                load_slot_main(2)
                load_slot_main(3)
                load_slot_rest(2)
                load_slot_rest(3)
            mel_glu(b, 0)
            if b < 3:
                mel_cast(b + 1, 0)
            mel_glu(b, 1)
            if b < 3:
                mel_cast(b + 1, 1)
            if b == 3:
                conv0(2)
                conv0(3)
            ph_glu(b, (0, 1))
            if b < 3:
                ph_cast(b + 1, 0)
                ph_cast(b + 1, 1)
        # bridge 1: mel_out head start fills the wait for the last ph adds,
        # then dense dots+exp for (0,1)
        mo0 = melout_items(0)
        mo1 = melout_items(1)
        run(mo0[:4] + mo1[:3])
        attn_pre(0)
        attn_pre(1)
        run(rr(dots_items(0), dots_items(1), mo0[4:], mo1[3:]))
        run(ctx_items(0))
        # phase 2: small slots' GLU with scalar-free wedges (ctx of slot 1)
        work = list(ctx_items(1))
        nw = len(work)
        mel_cast(0, 2)
        mel_cast(0, 3)
        ph_cast(0, 2)
        ph_cast(0, 3)
        for b in range(4):
            mel_glu(b, 2)
            if b < 3:
                mel_cast(b + 1, 2)
            take(work, nw // 12)
            mel_glu(b, 3)
            if b < 3:
                mel_cast(b + 1, 3)
            take(work, nw // 12)
            ph_glu(b, (2, 3))
            if b < 3:
                ph_cast(b + 1, 2)
                ph_cast(b + 1, 3)
            take(work, nw // 12)
        run(work)
        # phase 3: small slots' attention tail
        attn_pre(2)
        attn_pre(3)
        run(rr(dots_items(2), dots_items(3), melout_items(2), melout_items(3)))
        run(rr(ctx_items(2), ctx_items(3)))

    if not nc.is_finalized():
        nc.finalize()
    return nc


def _get_program(S_pad, L, SL, LO, LOPH):
    key = (S_pad, L, SL, LO, LOPH)
    if key not in _prog_cache:
        _prog_cache[key] = _build_program(S_pad, L, SL, LO, LOPH)
    return _prog_cache[key]


LAST_RESULTS = None


def _install_ntff_hook():
    """Provide antenv.axon_hooks (missing in this image) so trace=True works."""
    import sys
    import types
    import ctypes
    import contextlib
    if "antenv.axon_hooks" in sys.modules:
        return
    try:
        import antenv
    except ImportError:
        return
    mod = types.ModuleType("antenv.axon_hooks")
    state = {}
    mod.set_axon_ntff_profile_hook = lambda h: state.__setitem__("h", h)
    mod.get_axon_ntff_profile_hook = lambda: state.get("h")
    sys.modules["antenv.axon_hooks"] = mod
    antenv.axon_hooks = mod
    so_path = "/opt/axon/libaxon_pjrt.so"
    if not os.path.exists(so_path):
        return
    lib = ctypes.CDLL(so_path)
    if not hasattr(lib, "axon_start_nrt_profile"):
        return
    lib.axon_start_nrt_profile.argtypes = [ctypes.POINTER(ctypes.c_int64),
                                           ctypes.c_size_t]
    lib.axon_start_nrt_profile.restype = ctypes.c_int64
    lib.axon_stop_nrt_profile.argtypes = [ctypes.c_char_p]
    lib.axon_stop_nrt_profile.restype = ctypes.c_int64

    @contextlib.contextmanager
    def _hook(output_dir, device_ids):
        import jax
        jax.devices()
        if device_ids:
            ids = (ctypes.c_int64 * len(device_ids))(*device_ids)
            rc = lib.axon_start_nrt_profile(ids, len(device_ids))
        else:
            rc = lib.axon_start_nrt_profile(None, 0)
        if rc != 0:
            raise RuntimeError(f"axon_start_nrt_profile rc={rc}")
        try:
            yield
        finally:
            n = lib.axon_stop_nrt_profile(str(output_dir).encode())
            print(f"ntff profile: {n} file(s) -> {output_dir}")

    mod.set_axon_ntff_profile_hook(_hook)


def kernel(mels, phonemes, mel_lens, phoneme_lens, embedding,
           mel_conv_w, mel_conv_b, ph_w, ph_b, mel_w, mel_b):
    global LAST_RESULTS
    from concourse.bass_utils import run_bass_kernel_spmd

    mels = np.asarray(mels)
    assert mels.shape == (B, T_MEL, MEL_D), mels.shape
    max_pl = int(np.max(np.asarray(phoneme_lens)))
    S_pad = 512 if max_pl <= 511 else 640

    in_maps, perm, L, SL, LO, LOPH = _host_prep(
        np.asarray(mels), np.asarray(phonemes), np.asarray(mel_lens),
        np.asarray(phoneme_lens), np.asarray(embedding),
        np.asarray(mel_conv_w), np.asarray(mel_conv_b),
        np.asarray(ph_w), np.asarray(ph_b),
        np.asarray(mel_w), np.asarray(mel_b), S_pad)

    nc = _get_program(S_pad, L, SL, LO, LOPH)
    trace = bool(int(os.environ.get("KERNEL_TRACE", "0")))
    if trace:
        _install_ntff_hook()
    res = run_bass_kernel_spmd(nc, in_maps, core_ids=list(range(N_CORES)),
                               trace=trace,
                               tmpdir=os.environ.get("KERNEL_TRACE_DIR"))
    LAST_RESULTS = res
    out = np.empty((B, T_MEL, 512), np.float32)
    Tb = [min(T_MEL, _ceil(L[j] + 2, 128)) for j in range(SPC)]
    for c in range(N_CORES):
        for j in range(SPC):
            bi = int(perm[8 * j + c])
            out[bi] = np.asarray(res.results[c]["out"][j], dtype=np.float32)
            if Tb[j] < T_MEL:
                out[bi, Tb[j]:, :256] = 0.0
                out[bi, Tb[j]:, 256:] = out[bi, L[j] + 1, 256:]
    return out


# revision 20
# speedup vs baseline: 1.2316x; 1.0106x over previous
"""Trainium2 Bass kernel for nn_AligningModel (mel/phoneme GLU encoders + soft attention).

Strategy (v2):
  - Data-parallel over batch: 32 samples -> 8 cores x 4 slots, length-sorted so
    each slot's compile-time bound is tight (slot j holds sorted ranks 8j..8j+7).
  - bf16 datapath everywhere; fp8e4m3 DoubleRow matmuls (256-contract/pass) for
    the mel GLU gate path and the entire phoneme encoder (error-tolerant paths,
    validated via numpy: ~7e-3 final rel err vs 2e-2 gate). Mel a-path stays bf16.
  - Ping-pong y tiles per GLU block: conv reads the old tile, the residual add
    writes the new one -> no masked input copy.  Masking is a narrow post-mask
    on cols [min_len_in_group+1, W+2) only (cols below are valid in all cores).
  - All conv weights SBUF-resident (bf16/fp8, ~3.5MB), DMA'd once per-block so
    block 0's weights land before the first GLU needs them.
  - Weight reuse: mel convs stream chunk-pairs per stationary load; phoneme
    convs stream both slots of a pair per load (hides 256-col DoubleRow LDW).
  - Scale folding: sqrt(0.5)^b folded into g-path conv weights; softmax uses
    logits = 2*C^8*dots - C^8*ph_sq (mel_sq dropped: softmax-invariant),
    no max-subtraction, phoneme -1e9 mask folded into per-partition exp bias.
  - Z (softmax denominator) via ones-columns appended to time-major ph tiles.
  - Attention emitted interleaved across slot pairs (PE dots ping-pong with
    scalar exp) with mel_out identity-matmuls as PE filler to stay HAM-warm.
"""

import os
import numpy as np
import ml_dtypes

B = 32
N_CORES = 8
SPC = 4           # samples (slots) per core
T_MEL = 2000
MEL_D = 80
D = 256
C = float(np.sqrt(0.5))
C4 = 0.25         # C**4 exact
C8 = 0.0625       # C**8 exact

BF = ml_dtypes.bfloat16
F8 = ml_dtypes.float8_e4m3

_prog_cache = {}


def _chunks(total, cap=512):
    out = []
    off = 0
    while off < total:
        w = min(cap, total - off)
        out.append((off, w))
        off += w
    return out


def _pairs(lst):
    return [lst[i:i + 2] for i in range(0, len(lst), 2)]


def _ceil(x, m):
    return -(-x // m) * m


def _host_prep(mels, phonemes, mel_lens, phoneme_lens, embedding,
               mel_conv_w, mel_conv_b, ph_w, ph_b, mel_w, mel_b, S_pad):
    """Build the per-core input maps (numpy only). Returns (in_maps, perm,
    L, SL, LO, LOPH) where perm[8*j + c] = original sample index in core c
    slot j."""
    f32 = np.float32
    SP2 = S_pad + 2

    order = np.argsort(-np.asarray(mel_lens), kind="stable")
    perm = np.asarray(order)
    L = tuple(int(mel_lens[perm[8 * j]]) for j in range(SPC))
    SL = tuple(int(max(phoneme_lens[perm[8 * j + c]] for c in range(8)))
               for j in range(SPC))
    LO = tuple(int(min(mel_lens[perm[8 * j + c]] for c in range(8)))
               for j in range(SPC))
    LOPH = tuple(int(min(phoneme_lens[perm[8 * j + c]] for c in range(8)))
                 for j in range(SPC))

    assert not np.any(mel_conv_b) and not np.any(mel_b) and not np.any(ph_b), \
        "nonzero conv biases not supported by this kernel variant"

    # conv0 weights: [i, k, o] bf16
    w0h = np.ascontiguousarray(
        np.transpose(mel_conv_w.astype(f32), (1, 2, 0))).astype(BF)

    scale = (C ** np.arange(4, dtype=np.float64)).astype(f32)

    def enc_layout(w4):
        # w4: [4, O, 256, 3] -> [128(ci), 4(b), 3(k), 2(icb), O]
        t = np.transpose(w4, (2, 0, 3, 1))          # [i, b, k, o]
        t = t.reshape(2, 128, 4, 3, w4.shape[1])    # [icb, ci, b, k, o]
        return np.ascontiguousarray(np.transpose(t, (1, 2, 3, 0, 4)))

    wa = enc_layout(mel_w[:, :256].astype(f32)).astype(BF)
    wg = enc_layout(mel_w[:, 256:].astype(f32)
                    * scale[:, None, None, None]).astype(F8)
    wpf = ph_w.astype(f32).copy()
    wpf[:, 256:] *= scale[:, None, None, None]
    wp = enc_layout(wpf).astype(F8)

    id1 = np.eye(128, dtype=f32).astype(BF)
    idc4 = (C4 * np.eye(128, dtype=f32)).astype(BF)

    shared = {"w0": w0h, "wam": wa, "wgm": wg, "wph": wp,
              "id1": id1, "idc4": idc4}

    ar = np.arange(T_MEL)
    ars = np.arange(S_pad)
    in_maps = []
    for c in range(N_CORES):
        idx = [int(perm[8 * j + c]) for j in range(SPC)]
        m = dict(shared)
        mcm = np.zeros((SPC, MEL_D, T_MEL + 4), BF)
        vm = np.zeros((SPC, T_MEL + 4), BF)
        zph = np.zeros((SPC, 2, 128, S_pad + 4), BF)
        vph = np.zeros((SPC, S_pad + 4), BF)
        mv = np.full((SPC, S_pad), -1e9, f32)
        for j, b in enumerate(idx):
            mcm[j, :, 2:T_MEL + 2] = np.asarray(mels[b], f32).T.astype(BF)
            vm[j, 2:T_MEL + 2] = (ar < int(mel_lens[b])).astype(BF)
            pl = int(phoneme_lens[b])
            ph_pad = np.concatenate([[0], np.asarray(phonemes[b], np.int64)])[:S_pad]
            e = embedding[ph_pad].astype(f32)
            valid = (ars[:len(e)] <= pl)
            e[~valid] = 0.0
            zph[j, :, :, 2:2 + len(e)] = e.T.reshape(2, 128, len(e)).astype(BF)
            vph[j, 2:2 + len(e)] = valid.astype(BF)
            mv[j, :len(e)][valid] = 0.0
        m["mels_cm"] = mcm
        m["valid_mel"] = vm
        m["zph0"] = zph
        m["valid_ph"] = vph
        m["mvec"] = mv
        in_maps.append(m)
    return in_maps, perm, L, SL, LO, LOPH


def _build_program(S_pad, L, SL, LO, LOPH):
    from contextlib import ExitStack
    import concourse.bass as bass
    import concourse.bacc as bacc
    import concourse.tile as tile
    from concourse import mybir

    f32 = mybir.dt.float32
    bf16 = mybir.dt.bfloat16
    f8 = mybir.dt.float8e4
    AF = mybir.ActivationFunctionType
    ALU = mybir.AluOpType
    AX = mybir.AxisListType
    DR = mybir.MatmulPerfMode.DoubleRow
    SZ = S_pad + 4                 # z tile width (data at col offset 2)
    SZQ = _ceil(SZ, 16)

    # per-slot compile-time bounds
    W = [min(T_MEL, _ceil(L[j] + 2, 4)) for j in range(SPC)]       # mel conv cols
    Tb = [min(T_MEL, _ceil(L[j] + 2, 128)) for j in range(SPC)]    # attn rows
    SW = [min(S_pad, _ceil(SL[j] + 2, 4)) for j in range(SPC)]     # ph conv cols
    NSB = [min(S_pad // 128, _ceil(SL[j] + 2, 128) // 128) for j in range(SPC)]
    WQ = [_ceil(W[j] + 4, 16) for j in range(SPC)]
    YW = [Tb[j] + 4 for j in range(SPC)]
    mel_chunks = [_chunks(W[j]) for j in range(SPC)]
    ph_chunks = [_chunks(SW[j]) for j in range(SPC)]
    dot_chunks = [_chunks(Tb[j]) for j in range(SPC)]
    # narrow post-mask regions (first possibly-invalid data col .. end of data)
    MLO = [min(LO[j] + 2, W[j] + 2) for j in range(SPC)]
    PLO = [min(LOPH[j] + 3, SW[j] + 2) for j in range(SPC)]

    nc = bacc.Bacc()
    t_mcm = nc.dram_tensor("mels_cm", [SPC, MEL_D, T_MEL + 4], bf16, kind="ExternalInput")
    t_vm = nc.dram_tensor("valid_mel", [SPC, T_MEL + 4], bf16, kind="ExternalInput")
    t_zph = nc.dram_tensor("zph0", [SPC, 2, 128, SZ], bf16, kind="ExternalInput")
    t_vph = nc.dram_tensor("valid_ph", [SPC, SZ], bf16, kind="ExternalInput")
    t_mv = nc.dram_tensor("mvec", [SPC, S_pad], f32, kind="ExternalInput")
    t_w0 = nc.dram_tensor("w0", [MEL_D, 3, 256], bf16, kind="ExternalInput")
    t_wam = nc.dram_tensor("wam", [128, 4, 3, 2, 256], bf16, kind="ExternalInput")
    t_wgm = nc.dram_tensor("wgm", [128, 4, 3, 2, 256], f8, kind="ExternalInput")
    t_wph = nc.dram_tensor("wph", [128, 4, 3, 2, 512], f8, kind="ExternalInput")
    t_id1 = nc.dram_tensor("id1", [128, 128], bf16, kind="ExternalInput")
    t_idc4 = nc.dram_tensor("idc4", [128, 128], bf16, kind="ExternalInput")
    t_out = nc.dram_tensor("out", [SPC, T_MEL, 512], bf16, kind="ExternalOutput")

    def bcast(ap, parts):
        return bass.AP(tensor=ap.tensor, offset=ap.offset,
                       ap=[[0, parts]] + list(ap.ap))

    def pbcast(ap):
        # [128, w] -> [128, 2, w] broadcasting over the plane dim
        a = list(ap.ap)
        return bass.AP(tensor=ap.tensor, offset=ap.offset,
                       ap=[list(a[0]), [0, 2], list(a[1])])

    with tile.TileContext(nc) as tc, ExitStack() as ctx:
        wconst = ctx.enter_context(tc.tile_pool(name="wconst", bufs=1))
        state = ctx.enter_context(tc.tile_pool(name="state", bufs=1))
        qpool = ctx.enter_context(tc.tile_pool(name="q", bufs=2))
        mpool = ctx.enter_context(tc.tile_pool(name="mcm", bufs=2))
        sgpool = ctx.enter_context(tc.tile_pool(name="sig", bufs=8))
        epool = ctx.enter_context(tc.tile_pool(name="ets", bufs=4))
        ztpool = ctx.enter_context(tc.tile_pool(name="ztm", bufs=4))
        spool = ctx.enter_context(tc.tile_pool(name="small", bufs=4))
        opool = ctx.enter_context(tc.tile_pool(name="oc", bufs=4))
        ppsum = ctx.enter_context(tc.tile_pool(name="pconv", bufs=6, space="PSUM"))
        tpsum = ctx.enter_context(tc.tile_pool(name="ptp", bufs=2, space="PSUM"))

        # ---- constants (block-split so blk 0 lands first) ----
        w0_t = wconst.tile([MEL_D, 3, 256], bf16, tag="w0")
        nc.scalar.dma_start(out=w0_t[:], in_=t_w0[:])
        id1_t = wconst.tile([128, 128], bf16, tag="id1")
        nc.scalar.dma_start(out=id1_t[:], in_=t_id1[:])
        idc4_t = wconst.tile([128, 128], bf16, tag="idc4")
        nc.scalar.dma_start(out=idc4_t[:], in_=t_idc4[:])
        wam_t = wconst.tile([128, 4, 3, 2, 256], bf16, tag="wam")
        wgm_t = wconst.tile([128, 4, 3, 2, 256], f8, tag="wgm")
        wph_t = wconst.tile([128, 4, 3, 2, 512], f8, tag="wph")

        def load_weights():
            for k in range(3):
                nc.sync.dma_start(out=wam_t[:, 0, k], in_=t_wam[:, 0, k])
            for k in range(3):
                nc.sync.dma_start(out=wgm_t[:, 0, k], in_=t_wgm[:, 0, k])
            nc.sync.dma_start(out=wph_t[:, 0], in_=t_wph[:, 0])
            nc.sync.dma_start(out=wam_t[:, 1:4], in_=t_wam[:, 1:4])
            nc.sync.dma_start(out=wgm_t[:, 1:4], in_=t_wgm[:, 1:4])
            nc.sync.dma_start(out=wph_t[:, 1:4], in_=t_wph[:, 1:4])

        ys = {}
        zs = {}
        mcs = {}
        vbs = {}
        vps = {}

        MCQ = {0: nc.scalar, 1: nc.gpsimd, 2: nc.scalar, 3: nc.gpsimd}

        def load_slot_main(s):
            # mel input first on its queue -- everything else queues behind
            mc = mpool.tile([MEL_D, W[s] + 4], bf16, tag=f"mcm{s}", name="mcm",
                            bufs=1)
            MCQ[s].dma_start(out=mc[:], in_=t_mcm[s, :, 0:W[s] + 4])
            mcs[s] = mc
            yt = [state.tile([128, 2, YW[s]], bf16, tag=f"y{s}_{i}", name="y")
                  for i in range(2)]
            for i in range(2):
                nc.vector.memset(yt[i][:, :, 0:2], 0.0)
                nc.vector.memset(yt[i][:, :, 2 + W[s]:YW[s]], 0.0)
            ys[s] = yt

        def load_slot_rest(s):
            zt = [state.tile([128, 2, SZ], bf16, tag=f"z{s}_{i}", name="z")
                  for i in range(2)]
            ZQ = {0: nc.scalar, 1: nc.gpsimd, 2: nc.scalar, 3: nc.gpsimd}
            ZQ[s].dma_start(out=zt[0][:],
                            in_=t_zph[s].rearrange("c p w -> p c w"))
            mw = W[s] + 2 - MLO[s]
            vb = wconst.tile([128, mw], bf16, tag=f"vm{s}", name="vm")
            nc.gpsimd.dma_start(out=vb[:], in_=bcast(t_vm[s, MLO[s]:W[s] + 2], 128))
            vbs[s] = vb
            pw = SW[s] + 2 - PLO[s]
            vp = wconst.tile([128, pw], bf16, tag=f"vp{s}", name="vp")
            nc.gpsimd.dma_start(out=vp[:], in_=bcast(t_vph[s, PLO[s]:SW[s] + 2], 128))
            vps[s] = vp
            nc.vector.memset(zt[1][:, :, 0:2], 0.0)
            nc.vector.memset(zt[1][:, :, 2 + SW[s]:SZ], 0.0)
            zs[s] = zt

        def mel_mask(s, dst):
            mw = W[s] + 2 - MLO[s]
            nc.gpsimd.tensor_mul(out=dst[:, :, MLO[s]:W[s] + 2],
                                 in0=dst[:, :, MLO[s]:W[s] + 2],
                                 in1=pbcast(vbs[s][:, 0:mw]))

        def ph_mask(s, dst):
            pw = SW[s] + 2 - PLO[s]
            nc.gpsimd.tensor_mul(out=dst[:, :, PLO[s]:SW[s] + 2],
                                 in0=dst[:, :, PLO[s]:SW[s] + 2],
                                 in1=pbcast(vps[s][:, 0:pw]))

        def conv0(s):
            mc = mcs[s]
            y0 = ys[s][0]
            for (off, n) in mel_chunks[s]:
                for ocb in range(2):
                    pi = ppsum.tile([128, 512], f32, tag="cps", name="cps")
                    for k in range(3):
                        nc.tensor.matmul(pi[:, :n],
                                         w0_t[:, k, 128 * ocb:128 * ocb + 128],
                                         mc[:, off + 1 + k:off + 1 + k + n],
                                         start=(k == 0), stop=(k == 2))
                    nc.scalar.copy(out=y0[:, ocb, off + 2:off + 2 + n],
                                   in_=pi[:, :n])
            mel_mask(s, y0)

        def cast_pieces(dst_q, src_y, chunks, wfull):
            # fp8 cast in chunk-aligned pieces so g-convs start early.
            prev = 0
            for i, (off, n) in enumerate(chunks):
                hi = min(off + n + 3, wfull)
                if hi > prev:
                    nc.vector.tensor_copy(out=dst_q[:, :, prev:hi],
                                          in_=src_y[:, :, prev:hi])
                prev = hi

        ymqs = {}

        def mel_cast(b, s):
            y_old = ys[s][b % 2]
            ymq = qpool.tile([128, 2, max(WQ)], f8, tag="qm", name="ymq")
            cast_pieces(ymq, y_old, mel_chunks[s], W[s] + 4)
            ymqs[s] = ymq

        def mel_glu(b, s):
            y_old = ys[s][b % 2]
            y_new = ys[s][(b + 1) % 2]
            ymq = ymqs[s]
            chunks = mel_chunks[s]
            for oco in range(2):
                def g_phase():
                    pg = {}
                    for (off, n) in chunks:
                        pg[off] = ppsum.tile([128, 512], f32, tag="cps",
                                             name="cps")
                    for k in range(3):
                        wsl = wgm_t[:, b, k, :, 128 * oco:128 * oco + 128]
                        for (off, n) in chunks:
                            nc.tensor.matmul(pg[off][:, :n], wsl,
                                             ymq[:, :, off + 1 + k:off + 1 + k + n],
                                             start=(k == 0), stop=(k == 2),
                                             perf_mode=DR)
                    sigs = {}
                    for (off, n) in chunks:
                        sig = sgpool.tile([128, 512], bf16, tag="sig", name="sig")
                        nc.scalar.activation(out=sig[:, :n], in_=pg[off][:, :n],
                                             func=AF.Sigmoid)
                        sigs[off] = sig
                    return sigs

                def a_phase():
                    pa = {}
                    for (off, n) in chunks:
                        pa[off] = ppsum.tile([128, 512], f32, tag="cps",
                                             name="cps")
                    for k in range(3):
                        for icb in range(2):
                            wsl = wam_t[:, b, k, icb, 128 * oco:128 * oco + 128]
                            st = (k == 0 and icb == 0)
                            sp = (k == 2 and icb == 1)
                            for (off, n) in chunks:
                                nc.tensor.matmul(pa[off][:, :n], wsl,
                                                 y_old[:, icb, off + 1 + k:off + 1 + k + n],
                                                 start=st, stop=sp)
                    return pa

                if b == 0:
                    pa = a_phase()
                    sigs = g_phase()
                else:
                    sigs = g_phase()
                    pa = a_phase()
                for (off, n) in chunks:
                    nc.vector.tensor_mul(out=sigs[off][:, :n], in0=pa[off][:, :n],
                                         in1=sigs[off][:, :n])
                    nc.vector.tensor_add(out=y_new[:, oco, off + 2:off + 2 + n],
                                         in0=sigs[off][:, :n],
                                         in1=y_old[:, oco, off + 2:off + 2 + n])
            if b < 3:
                mel_mask(s, y_new)

        zqs = {}

        def ph_cast(b, s):
            z_old = zs[s][b % 2]
            q = qpool.tile([128, 2, SZQ], f8, tag="qp", name="zq")
            nc.gpsimd.tensor_copy(out=q[:, :, 0:SW[s] + 3],
                                  in_=z_old[:, :, 0:SW[s] + 3])
            zqs[s] = q

        def ph_glu(b, ss):
            zq = zqs
            for oco in range(2):
                pp = {}
                for path in range(2):
                    for s in ss:
                        pp[(s, path)] = ppsum.tile([128, 512], f32, tag="cps",
                                                   name="cps")
                    col0 = 256 * path + 128 * oco
                    for k in range(3):
                        wsl = wph_t[:, b, k, :, col0:col0 + 128]
                        for s in ss:
                            (off, n) = ph_chunks[s][0]
                            nc.tensor.matmul(pp[(s, path)][:, :n], wsl,
                                             zq[s][:, :, off + 1 + k:off + 1 + k + n],
                                             start=(k == 0), stop=(k == 2),
                                             perf_mode=DR)
                for s in ss:
                    n = ph_chunks[s][0][1]
                    z_old = zs[s][b % 2]
                    z_new = zs[s][(b + 1) % 2]
                    sig = sgpool.tile([128, 512], bf16, tag="sig", name="sig")
                    nc.scalar.activation(out=sig[:, :n], in_=pp[(s, 1)][:, :n],
                                         func=AF.Sigmoid)
                    nc.vector.tensor_mul(out=sig[:, :n], in0=pp[(s, 0)][:, :n],
                                         in1=sig[:, :n])
                    nc.vector.tensor_add(out=z_new[:, oco, 2:2 + n],
                                         in0=sig[:, :n],
                                         in1=z_old[:, oco, 2:2 + n])
            if b < 3:
                for s in ss:
                    ph_mask(s, zs[s][(b + 1) % 2])

        def melout_items(s):
            y4 = ys[s][0]

            def tile_job(tt):
                def go():
                    rows = min(128, Tb[s] - 128 * tt)
                    tp = tpsum.tile([128, 256], f32, tag="tp", name="tp")
                    for dcb in range(2):
                        nc.tensor.matmul(tp[:rows, 128 * dcb:128 * dcb + 128],
                                         y4[:, dcb, 2 + 128 * tt:2 + 128 * tt + rows],
                                         idc4_t[:],
                                         start=(dcb == 0), stop=(dcb == 1))
                    om = opool.tile([128, 256], bf16, tag="om", name="om")
                    nc.vector.tensor_copy(out=om[:rows], in_=tp[:rows])
                    nc.scalar.dma_start(
                        out=t_out[s, 128 * tt:128 * tt + rows, 0:256],
                        in_=om[:rows])
                return go

            return [tile_job(tt) for tt in range((Tb[s] + 127) // 128)]

        attn_state = {}

        def attn_pre(s):
            n_sb = NSB[s]
            mv_t = spool.tile([128, n_sb], f32, tag=f"mv{s}", name="mv", bufs=1)
            src = t_mv[s]
            nc.gpsimd.dma_start(out=mv_t[:], in_=bass.AP(
                tensor=src.tensor, offset=src.offset,
                ap=[[1, 128], [128, n_sb]]))
            z4 = zs[s][0]
            zts = []
            biases = []
            for sb in range(n_sb):
                ztm = ztpool.tile([128, 260], bf16, tag=f"zt{s}", name="ztm",
                                  bufs=n_sb)
                tp = tpsum.tile([128, 256], f32, tag="tp", name="tp")
                for dcb in range(2):
                    nc.tensor.matmul(tp[:, 128 * dcb:128 * dcb + 128],
                                     z4[:, dcb, 2 + 128 * sb:130 + 128 * sb],
                                     id1_t[:],
                                     start=(dcb == 0), stop=(dcb == 1))
                nc.vector.tensor_copy(out=ztm[:, 0:256], in_=tp[:])
                nc.vector.memset(ztm[:, 256:260], 4.0)
                sq = opool.tile([128, 256], f32, tag="sq", name="sq", bufs=2)
                nc.gpsimd.tensor_mul(out=sq[:], in0=ztm[:, 0:256],
                                     in1=ztm[:, 0:256])
                ph2 = spool.tile([128, 1], f32, tag="ph2", name="ph2")
                nc.vector.tensor_reduce(out=ph2[:], in_=sq[:], axis=AX.X,
                                        op=ALU.add)
                bias_sb = spool.tile([128, 1], f32, tag=f"bias{s}", name="bias",
                                     bufs=n_sb)
                nc.gpsimd.tensor_scalar(out=bias_sb[:], in0=ph2[:],
                                        scalar1=-C8, scalar2=mv_t[:, sb:sb + 1],
                                        op0=ALU.mult, op1=ALU.add)
                zts.append(ztm)
                biases.append(bias_sb)
            attn_state[s] = (zts, biases, [None] * n_sb)

        def dots_exp_item(s, sb, off, n):
            zts, biases, ets = attn_state[s]
            if ets[sb] is None:
                ets[sb] = epool.tile([128, Tb[s]], bf16, tag=f"et{s}",
                                     name="et", bufs=NSB[s])
            z4 = zs[s][0]
            y4 = ys[s][0]
            dp = ppsum.tile([128, 512], f32, tag="cps", name="dps")
            for dcb in range(2):
                nc.tensor.matmul(dp[:, :n],
                                 z4[:, dcb, 2 + 128 * sb:130 + 128 * sb],
                                 y4[:, dcb, 2 + off:2 + off + n],
                                 start=(dcb == 0), stop=(dcb == 1))
            nc.scalar.activation(out=ets[sb][:, off:off + n], in_=dp[:, :n],
                                 func=AF.Exp, bias=biases[sb], scale=2 * C8)

        def dots_items(s):
            def job(sb, off, n):
                return lambda: dots_exp_item(s, sb, off, n)
            return [job(sb, off, n)
                    for (off, n) in dot_chunks[s]
                    for sb in range(NSB[s])]

        def ctx_items(s):
            n_sb = NSB[s]

            def tile_job(tt):
                def go():
                    zts, biases, ets = attn_state[s]
                    rows = min(128, Tb[s] - 128 * tt)
                    cp = ppsum.tile([128, 512], f32, tag="cps", name="cxs")
                    for sb in range(n_sb):
                        nc.tensor.matmul(cp[:rows, 0:260],
                                         ets[sb][:, 128 * tt:128 * tt + rows],
                                         zts[sb][:],
                                         start=(sb == 0), stop=(sb == n_sb - 1))
                    rc = spool.tile([128, 1], f32, tag="rc", name="rc")
                    nc.vector.reciprocal(out=rc[:rows], in_=cp[:rows, 256:257])
                    oc = opool.tile([128, 256], bf16, tag="oc", name="oc")
                    nc.vector.tensor_scalar_mul(out=oc[:rows],
                                                in0=cp[:rows, 0:256],
                                                scalar1=rc[:rows])
                    nc.sync.dma_start(
                        out=t_out[s, 128 * tt:128 * tt + rows, 256:512],
                        in_=oc[:rows])
                return go

            return [tile_job(tt) for tt in range((Tb[s] + 127) // 128)]

        # ---- emission ----
        load_slot_main(0)
        load_slot_main(1)
        load_weights()
        load_slot_rest(0)
        load_slot_rest(1)
        conv0(0)
        conv0(1)

        def rr(*worklists):
            worklists = [list(w) for w in worklists]
            out = []
            i = 0
            while any(worklists):
                wl = worklists[i % len(worklists)]
                if wl:
                    out.append(wl.pop(0))
                i += 1
            return out

        def run(jobs):
            for j in jobs:
                j()

        def take(work, n):
            for _ in range(n):
                if work:
                    work.pop(0)()

        # phase 1: big slots' full GLU stack; casts emitted right after the
        # producing slot's pointwise so the next block never waits on them
        mel_cast(0, 0)
        mel_cast(0, 1)
        ph_cast(0, 0)
        ph_cast(0, 1)
        for b in range(4):
            if b == 1:
                load_slot_main(2)
                load_slot_main(3)
                load_slot_rest(2)
                load_slot_rest(3)
            mel_glu(b, 0)
            if b < 3:
                mel_cast(b + 1, 0)
            mel_glu(b, 1)
            if b < 3:
                mel_cast(b + 1, 1)
            ph_glu(b, (0, 1))
            if b < 3:
                ph_cast(b + 1, 0)
                ph_cast(b + 1, 1)
        # bridge 1: small-slot conv0 as filler, then dense dots+exp for (0,1)
        conv0(2)
        conv0(3)
        attn_pre(0)
        attn_pre(1)
        run(rr(dots_items(0), dots_items(1), melout_items(0),
               melout_items(1)))
        run(ctx_items(0))
        # phase 2: small slots' GLU with scalar-free wedges (ctx of slot 1)
        work = list(ctx_items(1))
        nw = len(work)
        mel_cast(0, 2)
        mel_cast(0, 3)
        ph_cast(0, 2)
        ph_cast(0, 3)
        for b in range(4):
            mel_glu(b, 2)
            if b < 3:
                mel_cast(b + 1, 2)
            take(work, nw // 12)
            mel_glu(b, 3)
            if b < 3:
                mel_cast(b + 1, 3)
            take(work, nw // 12)
            ph_glu(b, (2, 3))
            if b < 3:
                ph_cast(b + 1, 2)
                ph_cast(b + 1, 3)
            take(work, nw // 12)
        run(work)
        # phase 3: small slots' attention tail
        attn_pre(2)
        attn_pre(3)
        run(rr(dots_items(2), dots_items(3), melout_items(2), melout_items(3)))
        run(rr(ctx_items(2), ctx_items(3)))

    if not nc.is_finalized():
        nc.finalize()
    return nc


def _get_program(S_pad, L, SL, LO, LOPH):
    key = (S_pad, L, SL, LO, LOPH)
    if key not in _prog_cache:
        _prog_cache[key] = _build_program(S_pad, L, SL, LO, LOPH)
    return _prog_cache[key]


LAST_RESULTS = None


def _install_ntff_hook():
    """Provide antenv.axon_hooks (missing in this image) so trace=True works."""
    import sys
    import types
    import ctypes
    import contextlib
    if "antenv.axon_hooks" in sys.modules:
        return
    try:
        import antenv
    except ImportError:
        return
    mod = types.ModuleType("antenv.axon_hooks")
    state = {}
    mod.set_axon_ntff_profile_hook = lambda h: state.__setitem__("h", h)
    mod.get_axon_ntff_profile_hook = lambda: state.get("h")
    sys.modules["antenv.axon_hooks"] = mod
    antenv.axon_hooks = mod
    so_path = "/opt/axon/libaxon_pjrt.so"
    if not os.path.exists(so_path):
        return
    lib = ctypes.CDLL(so_path)
    if not hasattr(lib, "axon_start_nrt_profile"):
        return
    lib.axon_start_nrt_profile.argtypes = [ctypes.POINTER(ctypes.c_int64),
                                           ctypes.c_size_t]
    lib.axon_start_nrt_profile.restype = ctypes.c_int64
    lib.axon_stop_nrt_profile.argtypes = [ctypes.c_char_p]
    lib.axon_stop_nrt_profile.restype = ctypes.c_int64

    @contextlib.contextmanager
    def _hook(output_dir, device_ids):
        import jax
        jax.devices()
        if device_ids:
            ids = (ctypes.c_int64 * len(device_ids))(*device_ids)
            rc = lib.axon_start_nrt_profile(ids, len(device_ids))
        else:
            rc = lib.axon_start_nrt_profile(None, 0)
        if rc != 0:
            raise RuntimeError(f"axon_start_nrt_profile rc={rc}")
        try:
            yield
        finally:
            n = lib.axon_stop_nrt_profile(str(output_dir).encode())
            print(f"ntff profile: {n} file(s) -> {output_dir}")

    mod.set_axon_ntff_profile_hook(_hook)


def kernel(mels, phonemes, mel_lens, phoneme_lens, embedding,
           mel_conv_w, mel_conv_b, ph_w, ph_b, mel_w, mel_b):
    global LAST_RESULTS
    from concourse.bass_utils import run_bass_kernel_spmd

    mels = np.asarray(mels)
    assert mels.shape == (B, T_MEL, MEL_D), mels.shape
    max_pl = int(np.max(np.asarray(phoneme_lens)))
    S_pad = 512 if max_pl <= 511 else 640

    in_maps, perm, L, SL, LO, LOPH = _host_prep(
        np.asarray(mels), np.asarray(phonemes), np.asarray(mel_lens),
        np.asarray(phoneme_lens), np.asarray(embedding),
        np.asarray(mel_conv_w), np.asarray(mel_conv_b),
        np.asarray(ph_w), np.asarray(ph_b),
        np.asarray(mel_w), np.asarray(mel_b), S_pad)

    nc = _get_program(S_pad, L, SL, LO, LOPH)
    trace = bool(int(os.environ.get("KERNEL_TRACE", "0")))
    if trace:
        _install_ntff_hook()
    res = run_bass_kernel_spmd(nc, in_maps, core_ids=list(range(N_CORES)),
                               trace=trace,
                               tmpdir=os.environ.get("KERNEL_TRACE_DIR"))
    LAST_RESULTS = res
    out = np.empty((B, T_MEL, 512), np.float32)
    Tb = [min(T_MEL, _ceil(L[j] + 2, 128)) for j in range(SPC)]
    for c in range(N_CORES):
        for j in range(SPC):
            bi = int(perm[8 * j + c])
            out[bi] = np.asarray(res.results[c]["out"][j], dtype=np.float32)
            if Tb[j] < T_MEL:
                out[bi, Tb[j]:, :256] = 0.0
                out[bi, Tb[j]:, 256:] = out[bi, L[j] + 1, 256:]
    return out
